# revision 13
# baseline (speedup 1.0000x reference)
"""Trainium2 Bass kernel: per-tensor asymmetric int8 activation quantization
followed by a linear layer (y = quantize(x) @ W.T + bias).

Sharding (8 cores): 4-way over tokens x 2-way over out_features.
Each core receives:
  xTh  [D_IN/2, TOK_C]     fp16  phase-0 scan half (d_in rolled per g-shard)
  xP   [128, MB, KB, 128]  fp32  host-permuted x for contiguous main DMA
  wT   [D_IN, DOUT_C]      fp16  (W transposed, out_feature-sharded)
  bias [DOUT_C]            fp16
and produces y [TOK_C, DOUT_C] fp16 (host upcasts to fp32).

v3 schedule (from trace analysis of v1 @746us: PE idle until 186us waiting
on the quant scale, GEMM window 550us at ~2.08 rows/ns = the realistic fp16
PE rate; fp8 DoubleRow measured exactly 2x so any >=2-pass fp8 scheme loses
to single-pass fp16):
  - phase-0 x scans an fp16 copy (8 MiB) on the sync+gpsimd queues at full
    fabric rate; Vector reduces fp16 at 2x DVE rate. fp16 min/max shifts
    the scale by <=2.4e-4 relative (~1e-3 output rel err; budget 2e-2).
  - W streams on the scalar queue from t=0; mb0's matmuls consume the
    per-kb chunks in arrival order (no stall: supply outpaces demand).
  - a warm-up AllReduce early in phase-0 absorbs the CC stream's one-time
    trigger/setup latency; the real AllReduce payload is partition-
    replicated [128,2] so no partition broadcast is needed afterwards.
  - bias is added during PSUM eviction on Vector (no K=1 bias matmuls);
    eviction writes fp16 (exact range: |y|max ~13k << 65504).
  - quantize is chunked in halves so mb0's first matmuls start ~6us sooner.
"""

import sys

import numpy as np

try:  # the grading environment may or may not have concourse on sys.path
    import concourse  # noqa: F401
except ImportError:  # pragma: no cover
    sys.path.insert(0, "/opt/trn_rl_repo")

P = 128
MAGIC = 12582912.0  # 1.5 * 2**23: fp32 add/sub rounds to nearest-even integer
QMIN, QMAX = -128.0, 127.0

# Full-problem shape (hardcoded per contract; kernel() checks them)
B, S, D_IN, D_OUT = 4, 2048, 4096, 4096
R_SHARDS, G_SHARDS = 4, 2  # token shards x out_feature shards
N_CORES = 8
WARM_AR = False


def build_program(d_in, tok, dout, n_cores=N_CORES):
    """Emit the per-core SPMD program. Returns a compiled Bacc object."""
    from contextlib import ExitStack

    import concourse.bacc as bacc
    import concourse.tile as tile
    from concourse import bass_isa, mybir

    f32, f16 = mybir.dt.float32, mybir.dt.float16
    AF = mybir.ActivationFunctionType
    ALU = mybir.AluOpType
    AX = mybir.AxisListType

    assert d_in % (2 * P) == 0 and tok % P == 0
    KB, MB = d_in // P, tok // P
    KB0 = KB // 2  # phase-0 min/max half (the other half is on the g-sibling)
    NMM = min(512, dout)
    assert dout % NMM == 0
    NB = dout // NMM

    nc = bacc.Bacc(
        "TRN2",
        target_bir_lowering=False,
        debug=False,
        num_devices=n_cores,
        enable_asserts=False,
    )

    xTh = nc.dram_tensor("xTh", [d_in // 2, tok], f16, kind="ExternalInput").ap()
    xP = nc.dram_tensor("xP", [P, MB, KB, P], f32, kind="ExternalInput").ap()
    wT = nc.dram_tensor("wT", [d_in, dout], f16, kind="ExternalInput").ap()
    bias = nc.dram_tensor("bias", [dout], f16, kind="ExternalInput").ap()
    y = nc.dram_tensor("y", [tok, dout], f16, kind="ExternalOutput").ap()
    cc_in = nc.dram_tensor("cc_in", [P * 2], f32).ap()
    cc_out = nc.dram_tensor("cc_out", [P * 2], f32, addr_space="Shared").ap()
    if WARM_AR:
        cw_in = nc.dram_tensor("cw_in", [2], f32).ap()
        cw_out = nc.dram_tensor("cw_out", [2], f32, addr_space="Shared").ap()

    xh_view = xTh.rearrange("(kb p) t -> p kb t", p=P)  # [P, KB0, tok]
    w_view = wT.rearrange("(kb p) o -> p kb o", p=P)  # [P, KB, dout]
    cc_in2 = cc_in.rearrange("(p j) -> p j", p=P)
    cc_out2 = cc_out.rearrange("(p j) -> p j", p=P)

    with tile.TileContext(nc) as tc, ExitStack() as ctx:
        wpool = ctx.enter_context(tc.tile_pool(name="w", bufs=1))
        xpool = ctx.enter_context(tc.tile_pool(name="x", bufs=2))
        qpool = ctx.enter_context(tc.tile_pool(name="q", bufs=2))
        opool = ctx.enter_context(tc.tile_pool(name="o", bufs=2))
        hpool = ctx.enter_context(tc.tile_pool(name="h", bufs=2))
        spool = ctx.enter_context(tc.tile_pool(name="s", bufs=1))
        ppool = ctx.enter_context(tc.tile_pool(name="ps", bufs=2, space="PSUM"))

        # ---- warm-up collective: absorbs the CC stream's one-time setup
        # latency so the real AllReduce later is latency-only. No trailing
        # wait; the real AR serializes behind it on the CC stream.
        if WARM_AR:
            wsc = spool.tile([1, 2], f32)
            nc.vector.memset(wsc[:], 0.0)
            sem_w = nc.alloc_semaphore("ar_warm")
            with tc.tile_critical():
                nc.gpsimd.dma_start(cw_in[None, :], wsc[:]).then_inc(sem_w, 16)
                nc.gpsimd.wait_ge(sem_w, 16)
                nc.gpsimd.collective_compute(
                    "AllReduce",
                    ALU.max,
                    replica_groups=[list(range(n_cores))],
                    ins=[cw_in],
                    outs=[cw_out],
                )

        # ---- phase 0: min/max over this core's fp16 scan half on 2 DMA
        # queues. InstTensorReduce runs at 1x only, so accumulate running
        # elementwise max/min via tensor_tensor (fp16 -> 2x_1p DVE mode) and
        # do a single 1x reduce on each accumulator at the end.
        run_max = spool.tile([P, tok], f16)
        run_min = spool.tile([P, tok], f16)
        nc.vector.memset(run_max[:], -60000.0)
        nc.vector.memset(run_min[:], 60000.0)
        ph0_queues = [nc.sync, nc.gpsimd]
        for kb in range(KB0):
            x16 = hpool.tile([P, tok], f16)
            ph0_queues[kb % 2].dma_start(x16[:], xh_view[:, kb, :])
            nc.vector.tensor_tensor(run_max[:], run_max[:], x16[:], op=ALU.max)
            nc.vector.tensor_tensor(run_min[:], run_min[:], x16[:], op=ALU.min)

        # Resident weights stream on the scalar queue from t=0 (bias first:
        # the partition broadcast later must not wait behind 16 MiB of W);
        # supply (~150 GB/s) outpaces mb0's consumption (one kb per ~1us
        # starting only after the scale is known).
        bias_bc = wpool.tile([P, dout], f16)
        nc.scalar.dma_start(bias_bc[0:1, :], bias[None, :])
        w_sb = wpool.tile([P, KB, dout], f16)
        for kb in range(KB):
            nc.scalar.dma_start(w_sb[:, kb : kb + 1, :], w_view[:, kb : kb + 1, :])

        # x main tiles for mb0/mb1: prefetched during phase 0 (the rest are
        # issued inside the loop, after the collective, on the same queue).
        x_pre = []
        for mb in range(2):
            x_m = xpool.tile([P, KB * P], f32, tag="xm")
            nc.gpsimd.dma_start(
                x_m.rearrange("p (a b) -> p a b", b=P), xP[:, mb, :, :]
            )
            x_pre.append(x_m)

        # partition partials -> [P,2] replicated, AllReduce(max) of
        # [xmax, -xmin] with the payload already partition-replicated.
        pk = spool.tile([P, 2], f32)
        nc.vector.tensor_reduce(pk[:, 0:1], run_max[:], axis=AX.X, op=ALU.max)
        nc.vector.tensor_reduce(pk[:, 1:2], run_min[:], axis=AX.X, op=ALU.min)
        nc.vector.tensor_scalar_mul(pk[:, 1:2], pk[:, 1:2], -1.0)
        pkr = spool.tile([P, 2], f32)
        nc.gpsimd.partition_all_reduce(
            pkr[:], pk[:], channels=P, reduce_op=bass_isa.ReduceOp.max
        )

        sc = spool.tile([P, 2], f32)
        sem_in = nc.alloc_semaphore("ar_in")
        sem_cc = nc.alloc_semaphore("ar_cc")
        sem_out = nc.alloc_semaphore("ar_out")
        with tc.tile_critical():
            nc.gpsimd.dma_start(cc_in2, pkr[:]).then_inc(sem_in, 16)
            nc.gpsimd.wait_ge(sem_in, 16)
            nc.gpsimd.collective_compute(
                "AllReduce",
                ALU.max,
                replica_groups=[list(range(n_cores))],
                ins=[cc_in],
                outs=[cc_out],
            ).then_inc(sem_cc, 1)
            nc.gpsimd.wait_ge(sem_cc, 1)
            nc.gpsimd.dma_start(sc[:], cc_out2).then_inc(sem_out, 16)
            nc.gpsimd.wait_ge(sem_out, 16)

        # bias broadcast: emitted after the AR chain so the gpsimd engine
        # stream cannot head-of-line-block the collective; only needed by
        # the first eviction (~30us after the scale lands).
        nc.gpsimd.partition_broadcast(bias_bc[:], bias_bc[0:1, :], channels=P)

        # ---- scalar math on [P,1] lanes (already replicated; no broadcast)
        scr = spool.tile([P, 6], f32)
        rng, inv, nt, zp = (scr[:, i : i + 1] for i in range(4))
        bc = spool.tile([P, 2], f32)
        isc, mzp = bc[:, 0:1], bc[:, 1:2]
        nc.vector.tensor_add(rng, sc[:, 0:1], sc[:, 1:2])  # xmax - xmin
        nc.vector.reciprocal(inv, rng)
        nc.vector.tensor_scalar_mul(isc, inv, 255.0)  # 255/(xmax-xmin) ~ 1/scale
        nc.vector.tensor_mul(nt, sc[:, 1:2], isc)  # (-xmin)/scale
        # rne(nt); then zp = clip(-128 + rne(nt), -128, 127)
        nc.vector.tensor_scalar(zp, nt, MAGIC, -MAGIC, op0=ALU.add, op1=ALU.add)
        nc.vector.tensor_scalar(zp, zp, -128.0, -128.0, op0=ALU.add, op1=ALU.max)
        nc.vector.tensor_scalar_min(zp, zp, 127.0)
        nc.vector.tensor_scalar(mzp, zp, -1.0, MAGIC, op0=ALU.mult, op1=ALU.add)

        # ---- main loop: quantize + matmul per 128-token block ----
        QC = 4  # quantize chunks per block: first matmuls start ~3us sooner
        CW = KB // QC * P
        for mb in range(MB):
            if mb < 2:
                x_m = x_pre[mb]
            else:
                x_m = xpool.tile([P, KB * P], f32, tag="xm")
                nc.gpsimd.dma_start(
                    x_m.rearrange("p (a b) -> p a b", b=P), xP[:, mb, :, :]
                )
            x_m3 = x_m.rearrange("p (a b) -> p a b", b=P)  # [P, KB, P]
            q_m = qpool.tile([P, KB, P], f16)
            q_m2 = q_m.rearrange("p a b -> p (a b)")
            # chunked quantize: kb quarters so mb0's matmuls start sooner
            for h in range(QC):
                cs = slice(h * CW, (h + 1) * CW)
                # v = x*inv_scale + MAGIC (ACT); upper bits hold rne(x/scale)
                nc.scalar.activation(
                    x_m[:, cs], x_m[:, cs], AF.Copy, bias=MAGIC, scale=bc[:, 0:1]
                )
                # v - (MAGIC - zp) = rne(x/scale) + zp ; clamp low
                nc.vector.tensor_scalar(
                    x_m[:, cs], x_m[:, cs], bc[:, 1:2], QMIN,
                    op0=ALU.subtract, op1=ALU.max,
                )
                nc.vector.tensor_scalar(q_m2[:, cs], x_m[:, cs], QMAX, None, op0=ALU.min)

            psum = ppool.tile([P, dout], f32)
            for kb in range(KB):
                lhsT = q_m[:, kb, :]
                for n in range(NB):
                    nc.tensor.matmul(
                        psum[:, n * NMM : (n + 1) * NMM],
                        lhsT,
                        w_sb[:, kb, n * NMM : (n + 1) * NMM],
                        start=(kb == 0),
                        stop=(kb == KB - 1),
                    )
            o_m = opool.tile([P, dout], f16, tag="o_m")
            nc.vector.scalar_tensor_tensor(
                o_m[:], psum[:], 1.0, bias_bc[:], op0=ALU.mult, op1=ALU.add
            )
            # alternate y queues (sync is free after phase 0) for a short tail
            yq = nc.sync if mb % 2 == 0 else nc.gpsimd
            yq.dma_start(y[mb * P : (mb + 1) * P, :], o_m[:])

    nc.compile()
    _dedupe_ldweights(nc)
    return nc


def _dedupe_ldweights(nc):
    """Remove back-to-back InstLdweights with identical weight access patterns.

    bacc's matmul split emits one Ldweights per Matmult even when consecutive
    matmuls share the stationary operand (our 4 n-slices per k-block). The PE
    keeps the stationary operand loaded between matmuls, so a repeat load with
    the same AP is pure overhead. Only drop loads that carry no semaphore
    waits/updates.
    """
    from concourse import mybir

    for fn in nc.m.functions:
        for bb in fn.blocks:
            insts = bb.instructions
            keep = []
            last_ldw_key = None
            removed = 0
            for inst in insts:
                tname = type(inst).__name__
                if tname == "InstLdweights":
                    key = inst.concise()
                    if (
                        key == last_ldw_key
                        and not inst.has_wait()
                        and not inst.has_update()
                    ):
                        removed += 1
                        continue
                    last_ldw_key = key
                elif tname == "InstMatmult":
                    pass  # matmuls stream; they don't disturb loaded weights
                elif getattr(inst, "engine", None) == mybir.EngineType.PE and tname not in (
                    "InstEventSemaphore",
                    "InstNop",
                ):
                    # any other PE instruction: be conservative
                    last_ldw_key = None
                keep.append(inst)
            if removed:
                del insts[:]
                for inst in keep:
                    insts.append(inst)


def make_in_maps(x, weight, bias, r_shards=R_SHARDS, g_shards=G_SHARDS):
    """Host-side shard/layout prep. Returns (in_maps, tok_c, dout_c)."""
    x = np.asarray(x, dtype=np.float32)
    weight = np.asarray(weight, dtype=np.float32)
    bias = np.asarray(bias, dtype=np.float32)
    tok_tot = int(np.prod(x.shape[:-1]))
    d_in = x.shape[-1]
    d_out = weight.shape[0]
    tok_c = tok_tot // r_shards
    dout_c = d_out // g_shards
    kb, mb = d_in // P, tok_c // P

    xt = np.ascontiguousarray(x.reshape(tok_tot, d_in).T)  # [d_in, tok_tot]
    b16 = bias.astype(np.float16)
    # g=1 cores get the d_in axis rolled by half so the SPMD program's
    # phase-0 min/max pass (which always scans the first d_in/2 rows) covers
    # the other half of x on the sibling core. Contraction order is
    # irrelevant to the matmul as long as xT and wT are rolled identically.
    half = d_in // 2

    def _roll(a, g):
        return a if g % 2 == 0 else np.concatenate([a[half:], a[:half]], axis=0)

    w_sh = []
    for g in range(g_shards):
        wg = weight[g * dout_c : (g + 1) * dout_c, :].T  # [d_in, dout_c] fp32
        w_sh.append(np.ascontiguousarray(_roll(wg, g).astype(np.float16)))

    in_maps = []
    for c in range(r_shards * g_shards):
        r, g = divmod(c, g_shards)
        xr = _roll(xt[:, r * tok_c : (r + 1) * tok_c], g)  # [d_in, tok_c]
        # xP[p, m, k, t] = xr[k*P + p, m*P + t]: per-partition contiguous lines
        xp = np.ascontiguousarray(xr.reshape(kb, P, mb, P).transpose(1, 2, 0, 3))
        m = {
            "xTh": np.ascontiguousarray(xr[:half]).astype(np.float16),
            "xP": xp,
            "wT": w_sh[g],
            "bias": np.ascontiguousarray(b16[g * dout_c : (g + 1) * dout_c]),
        }
        in_maps.append(m)
    return in_maps, tok_c, dout_c


def assemble_output(results, out_shape, tok_c, dout_c, g_shards=G_SHARDS):
    d_out = out_shape[-1]
    tok_tot = int(np.prod(out_shape[:-1]))
    Y = np.empty((tok_tot, d_out), np.float32)
    for c, res in enumerate(results):
        r, g = divmod(c, g_shards)
        Y[r * tok_c : (r + 1) * tok_c, g * dout_c : (g + 1) * dout_c] = res[
            "y"
        ].astype(np.float32)
    return Y.reshape(out_shape)


_PROGRAM_CACHE = {}


def _get_program(d_in, tok_c, dout_c):
    key = (d_in, tok_c, dout_c)
    if key not in _PROGRAM_CACHE:
        _PROGRAM_CACHE[key] = build_program(d_in, tok_c, dout_c, N_CORES)
    return _PROGRAM_CACHE[key]


def kernel(x, weight, bias, trace=False, **_ignored):
    """Full-input entry point: shards across 8 NeuronCores, runs, gathers."""
    from concourse.bass_utils import run_bass_kernel_spmd

    assert x.shape == (B, S, D_IN) and weight.shape == (D_OUT, D_IN)
    in_maps, tok_c, dout_c = make_in_maps(x, weight, bias)
    nc = _get_program(D_IN, tok_c, dout_c)
    out = run_bass_kernel_spmd(nc, in_maps, list(range(N_CORES)), trace=trace)
    res = assemble_output(out.results, (B, S, D_OUT), tok_c, dout_c)
    if trace:
        return res, out
    return res
